# revision 35
# baseline (speedup 1.0000x reference)
"""Multi-head attention (B=2, S=2048, D=1024, H=16) on 8 TRN2 NeuronCores.

Sharding: tensor-parallel over heads. Core c owns heads {2c, 2c+1} (feature
columns [128c, 128c+128)). All matmul inputs bf16; psum accumulation fp32.

Per-core plan (engine-balanced around the ACT-engine exp floor):
  - Q^T/K^T projections feature-major [feat, tok]; bk dropped (cancels in
    softmax); 1/sqrt(dh) folded into Wq/bq on host.
  - V projected token-major [tok, feat] directly (no transposes); bv folded
    into the output-projection bias on host (bo' = bo + bv @ Wo).
  - scores per (batch, head): out [128 keys, 1024 q] psum, exp on ACT into
    bf16; attnV in [q, dh] layout (full PE utilization); denominator via
    N=1 matmuls against a ones vector; normalization via per-partition
    reciprocal + tensor_scalar_mul on DVE.
  - Token ownership: core c owns tokens [256c, 256c+256) of EACH batch.
    One AllToAll per batch (launched as soon as that batch's two heads
    finish), receiver-side dma_start_transpose to feature-major, output
    projection per batch overlaps the next batch's attention.
  - PE is in-order, so PE idle gaps during the ACT-bound attention are
    backfilled with fine-grained (per-matmul) filler units: the remaining
    projections and the batch-0 output projection.
"""
import sys
sys.path.insert(0, "/opt/trn_rl_repo")
from collections import deque
from contextlib import ExitStack

import numpy as np

import concourse.bass as bass
import concourse.bacc as bacc
import concourse.mybir as mybir
import concourse.tile as tile
from concourse.bass_utils import run_bass_kernel_spmd

N_CORES = 8
B, S, D = 2, 2048, 1024
T = B * S                  # 4096 tokens
H, DH = 16, 64
F = D // N_CORES           # 128 features per core (2 heads)
TPC = 256                  # tokens owned per (core, batch)
NT = 8                     # 512-token tiles
NKT = 16                   # key tiles of 128 per batch
NCC = 8                    # contraction chunks of 128

F32 = mybir.dt.float32
BF16 = mybir.dt.bfloat16
EXP = mybir.ActivationFunctionType.Exp

_cache = {}
_DEBUG_OT = False


class FillQueue:
    """FIFO of (pe_ns, flags, emit_fn) filler units, drained between attention
    iterations to backfill PE idle. drain_until(flag) force-emits everything
    up to and including the unit that provides `flag`."""

    def __init__(self):
        self.q = deque()
        self.flags = set()

    def add(self, ns, fn, flags=()):
        self.q.append((ns, tuple(flags), fn))

    def _pop(self):
        ns, flags, fn = self.q.popleft()
        fn()
        self.flags.update(flags)
        return ns

    def drain(self, budget_ns):
        while self.q and budget_ns > 0:
            budget_ns -= self._pop()

    def drain_until(self, flag):
        while flag not in self.flags:
            assert self.q, f"filler queue exhausted waiting for {flag}"
            self._pop()

    def drain_all(self):
        while self.q:
            self._pop()


def build_nc():
    nc = bacc.Bacc()
    # host-prepacked layouts (see _prep_inputs)
    x_e = nc.dram_tensor("xh", [128, NT * 4096], BF16, kind="ExternalInput")
    wq_e = nc.dram_tensor("wq", [128, D], BF16, kind="ExternalInput")
    wk_e = nc.dram_tensor("wk", [128, D], BF16, kind="ExternalInput")
    wv_e = nc.dram_tensor("wv", [128, D], BF16, kind="ExternalInput")
    bq_e = nc.dram_tensor("bq", [128, 1], F32, kind="ExternalInput")
    wo_e = nc.dram_tensor("wo", [128, NCC * D], BF16, kind="ExternalInput")
    bo2_e = nc.dram_tensor("bo2", [128, NCC], F32, kind="ExternalInput")
    id_e = nc.dram_tensor("ident", [128, 128], BF16, kind="ExternalInput")
    outT_e = nc.dram_tensor("outT", [D, 2 * TPC], F32, kind="ExternalOutput")
    dbg_e = nc.dram_tensor("dbg", [128, 2 * 2048], BF16,
                           kind="ExternalOutput") if _DEBUG_OT else None
    dbg2_e = nc.dram_tensor("dbg2", [128, 3 * T], BF16,
                            kind="ExternalOutput") if _DEBUG_OT else None

    with tile.TileContext(nc) as tc, ExitStack() as top:
        misc = top.enter_context(tc.tile_pool(name="misc", bufs=1))
        bq_sb = misc.tile([128, 1], F32)
        bo2_sb = misc.tile([128, NCC], F32)
        ones_sb = misc.tile([128, 1], BF16)
        id_sb = misc.tile([128, 128], BF16)
        nc.gpsimd.memset(ones_sb[:], 1.0)

        sb = top.enter_context(tc.tile_pool(name="sb", bufs=1))
        wq_sb = sb.tile([128, D], BF16, tag="wq")
        wk_sb = sb.tile([128, D], BF16, tag="wk")
        wv_sb = sb.tile([128, D], BF16, tag="wv")
        wo_sb = sb.tile([128, NCC * D], BF16, tag="wo")
        Qt = sb.tile([128, T], BF16, tag="Qt")      # [feat, tok]
        Kt = sb.tile([128, T], BF16, tag="Kt")
        xs = [sb.tile([128, 4096], BF16, tag=f"xs{t}", name=f"xs{t}")
              for t in range(NT)]
        V2 = [sb.tile([128, 512], BF16, tag=f"v2_{t}", name=f"v2_{t}")
              for t in range(NT)]                   # [tok128x4, feat]
        OT = [sb.tile([128, 2048], BF16, tag=f"ot{b}", name=f"ot{b}")
              for b in range(B)]                    # [q128, qt*128 + 64h + dh]
        InT = [[sb.tile([128, 1024], BF16, tag=f"in{b}{hf}",
                        name=f"in{b}{hf}") for hf in range(2)]
               for b in range(B)]                   # [feat128, 128cc + tok]
        Atm = [[sb.tile([128, 1024], BF16, tag=f"atm{b}{hf}",
                        name=f"atm{b}{hf}") for hf in range(2)]
               for b in range(B)]                   # token-major recv staging
        atp = top.enter_context(tc.tile_pool(name="atp", bufs=3))
        recp = top.enter_context(tc.tile_pool(name="recp", bufs=2))
        osbp = top.enter_context(tc.tile_pool(name="osbp", bufs=2))

        # PSUM: pair(sc + tail outproj) 2x[128,1024]=4 banks, ov 2, dn 1,
        # b512 (v-proj / interleaved qk-proj / outproj-b0 fillers) 1 => 8
        pairp = top.enter_context(tc.tile_pool(name="pairp", bufs=2, space="PSUM"))
        ovp = top.enter_context(tc.tile_pool(name="ovp", bufs=1, space="PSUM"))
        dnp_pool = top.enter_context(tc.tile_pool(name="dnp", bufs=1, space="PSUM"))
        b512 = top.enter_context(tc.tile_pool(name="b512", bufs=1, space="PSUM"))

        # Each batch's AllToAll is split into two half collectives of
        # [8, 128, 128]: core c owns tokens qt=c (half 0) and qt=8+c (half 1)
        # of each batch, so the lo half is complete as soon as the qh0 sweep
        # of the batch's last head finishes.
        dram = top.enter_context(tc.tile_pool(name="dram", bufs=1, space="DRAM"))
        a2a_in = [[dram.tile([N_CORES, 128, 128], BF16, tag=f"ain{b}{hf}",
                             name=f"ain{b}{hf}") for hf in range(2)]
                  for b in range(B)]
        a2a_out = [[dram.tile([N_CORES, 128, 128], BF16, tag=f"aout{b}{hf}",
                              name=f"aout{b}{hf}") for hf in range(2)]
                   for b in range(B)]

        # input DMAs, readiness-ordered on the SP queue
        nc.sync.dma_start(out=wq_sb[:], in_=wq_e[:])
        nc.sync.dma_start(out=xs[0][:, 0:2048], in_=x_e[:, 0:2048])
        nc.sync.dma_start(out=wk_sb[:], in_=wk_e[:])
        nc.sync.dma_start(out=xs[0][:, 2048:4096], in_=x_e[:, 2048:4096])
        nc.sync.dma_start(out=xs[1][:, 0:2048], in_=x_e[:, 4096:6144])
        nc.sync.dma_start(out=xs[1][:, 2048:4096], in_=x_e[:, 6144:8192])
        nc.sync.dma_start(out=bq_sb[:], in_=bq_e[:])
        nc.sync.dma_start(out=wv_sb[:], in_=wv_e[:])
        nc.sync.dma_start(out=bo2_sb[:], in_=bo2_e[:])
        nc.sync.dma_start(out=id_sb[:], in_=id_e[:])
        for t in range(2, NT):
            nc.sync.dma_start(out=xs[t][:], in_=x_e[:, 4096 * t:4096 * (t + 1)])
        nc.sync.dma_start(out=wo_sb[:], in_=wo_e[:])

        # ---------- projection emitters ----------
        def emit_pair(kind, ta, tb):
            """Blob form (pre-attention only): [128,1024] psum with proj of
            tiles ta,tb."""
            w_sb, dst = (wq_sb, Qt) if kind == "q" else (wk_sb, Kt)
            pt = pairp.tile([128, 1024], F32, tag="pair", name=f"p{kind}{ta}{tb}")
            for i, t in enumerate((ta, tb)):
                for cc in range(NCC):
                    nc.tensor.matmul(
                        pt[:, 512 * i:512 * (i + 1)],
                        w_sb[:, 128 * cc:128 * (cc + 1)],
                        xs[t][:, 512 * cc:512 * (cc + 1)],
                        start=(cc == 0), stop=(cc == NCC - 1))
            for i, t in enumerate((ta, tb)):
                sl = pt[:, 512 * i:512 * (i + 1)]
                if kind == "q":
                    nc.vector.tensor_scalar_add(
                        dst[:, 512 * t:512 * (t + 1)], sl, bq_sb[:])
                else:
                    nc.vector.tensor_copy(dst[:, 512 * t:512 * (t + 1)], sl)

        def add_qk_units(fill, kind, t, flags):
            """Fine-grained filler: Q or K projection of tile t on b512."""
            w_sb, dst = (wq_sb, Qt) if kind == "q" else (wk_sb, Kt)
            box = {}

            def mk(cc):
                def emit():
                    if cc == 0:
                        box["pt"] = b512.tile([128, 512], F32, tag="b512",
                                              name=f"s{kind}{t}")
                    nc.tensor.matmul(
                        box["pt"][:], w_sb[:, 128 * cc:128 * (cc + 1)],
                        xs[t][:, 512 * cc:512 * (cc + 1)],
                        start=(cc == 0), stop=(cc == NCC - 1))
                return emit

            for cc in range(NCC):
                fill.add(430, mk(cc))

            def fin():
                if kind == "q":
                    nc.vector.tensor_scalar_add(
                        dst[:, 512 * t:512 * (t + 1)], box["pt"][:], bq_sb[:])
                else:
                    nc.vector.tensor_copy(dst[:, 512 * t:512 * (t + 1)],
                                          box["pt"][:])
            fill.add(60, fin, flags)

        def add_v_units(fill, t, flags):
            box = {}

            def mk(j):
                def emit():
                    if j == 0:
                        box["vp"] = b512.tile([128, 512], F32, tag="b512",
                                              name=f"vps{t}")
                    for cc in range(NCC):
                        nc.tensor.matmul(
                            box["vp"][:, 128 * j:128 * (j + 1)],
                            xs[t][:, 512 * cc + 128 * j:512 * cc + 128 * (j + 1)],
                            wv_sb[:, 128 * cc:128 * (cc + 1)],
                            start=(cc == 0), stop=(cc == NCC - 1))
                return emit

            for j in range(4):
                fill.add(430, mk(j))

            def fin():
                nc.vector.tensor_copy(V2[t][:], box["vp"][:])
            fill.add(60, fin, flags)

        # ---------- output projection ----------
        # out columns: outT_e[:, 256b + 128hf + tok]; token = 128c + 1024hf
        def _outproj_chunk(b, hf, n, ops, osb):
            for cc in range(NCC):
                nc.tensor.matmul(
                    ops[:, 0:128],
                    wo_sb[:, D * cc + 128 * n:D * cc + 128 * (n + 1)],
                    InT[b][hf][:, 128 * cc:128 * (cc + 1)],
                    start=(cc == 0), stop=(cc == NCC - 1))
            nc.vector.tensor_scalar_add(
                osb[:, 128 * n:128 * (n + 1)], ops[:, 0:128], bo2_sb[:, n:n + 1])

        def _outproj_dma(b, hf, osb):
            nc.sync.dma_start(
                out=outT_e[:, 256 * b + 128 * hf:256 * b + 128 * (hf + 1)]
                    .rearrange("(n p) f -> p n f", p=128),
                in_=osb[:].rearrange("p (n f) -> p n f", n=NCC))

        def add_outproj_units(q_lo, q_hi, b):
            """Filler form for batch b on the b512 psum tag; hf=0 units go to
            q_lo, hf=1 (gated on the later collective half) to q_hi."""
            for hf, q in ((0, q_lo), (1, q_hi)):
                state = {}

                def mk(n, hf=hf, state=state):
                    def emit():
                        if n == 0:
                            state["osb"] = osbp.tile([128, 1024], F32, tag="osb",
                                                     name=f"osb{b}{hf}")
                        ops = b512.tile([128, 512], F32, tag="b512",
                                        name=f"ops{b}{hf}{n}")
                        _outproj_chunk(b, hf, n, ops, state["osb"])
                        if n == NCC - 1:
                            _outproj_dma(b, hf, state["osb"])
                    return emit

                for n in range(NCC):
                    q.add(500, mk(n))

        def emit_outproj_tail(b, hf):
            """Tail form: pair-pool tile per fout chunk (bufs=2 pipelines the
            psum WAR between a chunk's bias-add read and the next's matmuls)."""
            osb = osbp.tile([128, 1024], F32, tag="osbq", name=f"osbq{b}{hf}")
            for n in range(NCC):
                ops = pairp.tile([128, 1024], F32, tag="pair",
                                 name=f"opc{b}{hf}{n}")
                _outproj_chunk(b, hf, n, ops, osb)
            _outproj_dma(b, hf, osb)

        # ---------- attention block ----------
        def attention_block(b, h, fill: FillQueue, carry=None, mid_extra=None,
                            late_fill=None, budget_ns=420):
            """Emits scores+exp+attnV for (b, h). The final attnV flush and
            normalization are NOT emitted here; they are returned as a
            `finish` closure which the NEXT block runs (via `carry`) right
            after its first exp, so the next block's scores are already in
            flight on the in-order PE before the flush/norm chain."""
            hs = slice(64 * h, 64 * (h + 1))
            ov = ovp.tile([128, 1024], F32, tag="ov", name=f"ov{b}{h}")
            dn = dnp_pool.tile([128, 16], F32, tag="dn", name=f"dn{b}{h}")
            pending = None
            first = True

            def emit_attnv(kt, qh, at):
                # The simulator zeroes psum lazily at ZERO-REGION (2KB bank)
                # granularity on start=True, so each bank must be ONE
                # accumulation group: start only on the bank's first write
                # (later regions read pending-zero as 0), stop on its last.
                fill.drain_until(f"v{b}t{kt // 4}")
                t = 4 * b + kt // 4
                j = kt % 4
                for i in range(8):
                    qt = 8 * qh + i
                    nc.tensor.matmul(
                        ov[:, 64 * qt:64 * (qt + 1)],
                        at[:, 128 * i:128 * (i + 1)],
                        V2[t][:, 128 * j + 64 * h:128 * j + 64 * h + 64],
                        start=(kt == 0 and i == 0),
                        stop=(kt == NKT - 1 and i == 7),
                        skip_group_check=True)
                    nc.tensor.matmul(
                        dn[:, qt:qt + 1],
                        at[:, 128 * i:128 * (i + 1)],
                        ones_sb[:, 0:1],
                        start=(qh == 0 and kt == 0 and i == 0),
                        stop=(qh == 1 and kt == NKT - 1 and i == 7),
                        skip_group_check=True)

            rc = recp.tile([128, 16], F32, tag="rc", name=f"rc{b}{h}")

            def norm_half(hf):
                # normalize qt in [8hf, 8hf+8): reciprocal + broadcast multiply
                nc.vector.reciprocal(rc[:, 8 * hf:8 * (hf + 1)],
                                     dn[:, 8 * hf:8 * (hf + 1)])
                nc.vector.tensor_mul(
                    OT[b][:, 1024 * hf:1024 * (hf + 1)]
                        .rearrange("p (qt hh d) -> p qt hh d",
                                   hh=2, d=64)[:, :, h, :],
                    ov[:, 512 * hf:512 * (hf + 1)]
                        .rearrange("p (qt d) -> p qt d", d=64),
                    rc[:, 8 * hf:8 * (hf + 1)].to_broadcast([128, 8, 64]))

            for qh in range(2):
                fill.drain_until(f"q{b}{'lo' if qh == 0 else 'hi'}")
                for kt in range(NKT):
                    fill.drain_until(f"k{b}t{kt // 4}")
                    sc = pairp.tile([128, 1024], F32, tag="pair",
                                    name=f"sc{b}{h}{qh}{kt}")
                    for i in range(2):
                        q0 = 2048 * b + 1024 * qh + 512 * i
                        nc.tensor.matmul(
                            sc[:, 512 * i:512 * (i + 1)],
                            Kt[hs, 2048 * b + 128 * kt:2048 * b + 128 * (kt + 1)],
                            Qt[hs, q0:q0 + 512],
                            start=True, stop=True)
                    at = atp.tile([128, 1024], BF16, tag="at",
                                  name=f"at{b}{h}{qh}{kt}")
                    nc.scalar.activation(at[:], sc[:], EXP)
                    if first:
                        if carry is not None:
                            carry()
                        first = False
                    if pending is not None:
                        emit_attnv(*pending)
                        if pending[0] == NKT - 1 and pending[1] == 0:
                            # qh0 sweep flushed: lo-half norm (and a2a hooks)
                            norm_half(0)
                            if mid_extra is not None:
                                mid_extra()
                    pending = (kt, qh, at)
                    fill.drain(budget_ns)
                    if late_fill is not None and qh == 1:
                        late_fill.drain(budget_ns)

            def finish():
                emit_attnv(*pending)
                norm_half(1)
            return finish

        def emit_stage(b, hf):
            nc.sync.dma_start(
                out=a2a_in[b][hf][:].rearrange("r p f -> p r f"),
                in_=OT[b][:, 1024 * hf:1024 * (hf + 1)]
                    .rearrange("p (r f) -> p r f", r=N_CORES))

        def emit_coll(b, hf):
            nc.gpsimd.collective_compute(
                "AllToAll", mybir.AluOpType.bypass,
                ins=[a2a_in[b][hf][:].opt()],
                outs=[a2a_out[b][hf][:].opt()],
                replica_groups=[list(range(N_CORES))])

        def emit_recv_load(b, hf):
            # token-major load of the half: Atm[p=tok, 128r + f]
            nc.sync.dma_start(
                out=Atm[b][hf][:].rearrange("p (r f) -> p r f", r=NCC),
                in_=a2a_out[b][hf][:].rearrange("r t f -> t r f"))

        def emit_recv_transpose(b, hf):
            # PE transposes to feature-major InT[b][hf] (cols = 128cc + tok).
            # (dma_start_transpose is serialized against collectives by the
            # tile framework, so transpose on the PE instead.)
            tp = pairp.tile([128, 1024], BF16, tag="pair", name=f"tp{b}{hf}")
            for r in range(NCC):
                nc.tensor.transpose(tp[:, 128 * r:128 * (r + 1)],
                                    Atm[b][hf][:, 128 * r:128 * (r + 1)],
                                    id_sb[:])
            nc.vector.tensor_copy(InT[b][hf][:], tp[:])

        def add_recv_units(q, b, hf):
            q.add(100, lambda: emit_recv_load(b, hf))
            q.add(600, lambda: emit_recv_transpose(b, hf))

        # ================= schedule =================
        fill = FillQueue()
        fill.flags.update({"q0lo", "k0t0", "k0t1"})
        # pre-phase: one psum tile per (tensor, t); t0 first (xs0 halves),
        # t1 matmuls hide behind the xs1 DMA
        for t in (0, 1):
            for kind, w_sb in (("q", wq_sb), ("k", wk_sb)):
                pt = pairp.tile([128, 1024], F32, tag="pair",
                                name=f"pre{kind}{t}")
                for cc in range(NCC):
                    nc.tensor.matmul(
                        pt[:, 0:512],
                        w_sb[:, 128 * cc:128 * (cc + 1)],
                        xs[t][:, 512 * cc:512 * (cc + 1)],
                        start=(cc == 0), stop=(cc == NCC - 1))
                if kind == "q":
                    nc.vector.tensor_scalar_add(
                        Qt[:, 512 * t:512 * (t + 1)], pt[:, 0:512], bq_sb[:])
                else:
                    nc.vector.tensor_copy(
                        Kt[:, 512 * t:512 * (t + 1)], pt[:, 0:512])

        add_v_units(fill, 0, ["v0t0"])
        add_v_units(fill, 1, ["v0t1"])
        add_qk_units(fill, "k", 2, ["k0t2"])
        add_v_units(fill, 2, ["v0t2"])
        add_qk_units(fill, "k", 3, ["k0t3"])
        add_v_units(fill, 3, ["v0t3"])
        add_qk_units(fill, "q", 2, [])
        add_qk_units(fill, "q", 3, ["q0hi"])
        add_qk_units(fill, "q", 4, [])
        add_qk_units(fill, "q", 5, ["q1lo"])
        add_qk_units(fill, "k", 4, ["k1t0"])
        add_v_units(fill, 4, ["v1t0"])
        add_qk_units(fill, "k", 5, ["k1t1"])
        add_v_units(fill, 5, ["v1t1"])
        add_qk_units(fill, "k", 6, ["k1t2"])
        add_v_units(fill, 6, ["v1t2"])
        add_qk_units(fill, "k", 7, ["k1t3"])
        add_v_units(fill, 7, ["v1t3"])
        add_qk_units(fill, "q", 6, [])
        add_qk_units(fill, "q", 7, ["q1hi"])

        f00 = attention_block(0, 0, fill)

        def mid01():   # b0 lo-half complete once b0h1's qh0 sweep is normed
            emit_stage(0, 0)
            emit_coll(0, 0)
        f01 = attention_block(0, 1, fill, carry=f00, mid_extra=mid01)

        def carry10():
            f01()
            emit_stage(0, 1)
            emit_coll(0, 1)
        f10 = attention_block(1, 0, fill, carry=carry10)
        late = FillQueue()
        add_recv_units(fill, 0, 0)
        add_recv_units(late, 0, 1)
        add_outproj_units(fill, late, 0)

        def mid11():
            emit_stage(1, 0)
            emit_coll(1, 0)
        f11 = attention_block(1, 1, fill, carry=f10, mid_extra=mid11,
                              late_fill=late)
        fill.drain_all()
        late.drain_all()
        f11()
        emit_stage(1, 1)
        emit_coll(1, 1)
        for hf in range(2):
            emit_recv_load(1, hf)
            emit_recv_transpose(1, hf)
            emit_outproj_tail(1, hf)
        if _DEBUG_OT:
            for b in range(B):
                nc.sync.dma_start(out=dbg_e[:, 2048 * b:2048 * (b + 1)],
                                  in_=OT[b][:])
            nc.sync.dma_start(out=dbg2_e[:, 0:T], in_=Qt[:])
            nc.sync.dma_start(out=dbg2_e[:, T:2 * T], in_=Kt[:])
            for t in range(NT):
                nc.sync.dma_start(
                    out=dbg2_e[:, 2 * T + 512 * t:2 * T + 512 * (t + 1)],
                    in_=V2[t][:])

    nc.finalize()
    return nc


def _prep_inputs(x, Wq, bq, Wk, bk, Wv, bv, Wo, bo):
    import ml_dtypes
    bf16 = ml_dtypes.bfloat16
    scale = 1.0 / np.sqrt(DH)

    xf = np.asarray(x, np.float32).reshape(T, D)
    # xh[p, 4096t + 512cc + f] = xf[512t + f, 128cc + p]
    xh = np.ascontiguousarray(
        xf.reshape(NT, 512, NCC, 128).transpose(3, 0, 2, 1).reshape(128, NT * 4096)
    ).astype(bf16)

    def pack_w(W):  # [1024, 128] -> [128, 1024]: out[p, 128cc+f] = W[128cc+p, f]
        return np.ascontiguousarray(
            np.asarray(W, np.float32).reshape(NCC, 128, 128)
            .transpose(1, 0, 2).reshape(128, D))

    Wo64 = np.asarray(Wo, np.float64)
    bo_f = (np.asarray(bo, np.float64) +
            np.asarray(bv, np.float64) @ Wo64).astype(np.float32)
    bo2 = np.ascontiguousarray(bo_f.reshape(NCC, 128).T)
    # wo_sb[p, 1024cc + fo] = Wo[128cc + p, fo]
    wo_p = np.ascontiguousarray(
        np.asarray(Wo, np.float32).reshape(NCC, 128, D)
        .transpose(1, 0, 2).reshape(128, NCC * D)).astype(bf16)
    ident = np.eye(128, dtype=np.float32).astype(bf16)

    in_maps = []
    for c in range(N_CORES):
        fs = slice(F * c, F * (c + 1))
        in_maps.append({
            "xh": xh,
            "wq": pack_w(np.asarray(Wq, np.float32)[:, fs] * scale).astype(bf16),
            "wk": pack_w(np.asarray(Wk, np.float32)[:, fs]).astype(bf16),
            "wv": pack_w(np.asarray(Wv, np.float32)[:, fs]).astype(bf16),
            "bq": np.ascontiguousarray(
                (np.asarray(bq, np.float32)[fs] * scale)[:, None]),
            "wo": wo_p,
            "bo2": bo2,
            "ident": ident,
        })
    return in_maps


def kernel(x, Wq, bq, Wk, bk, Wv, bv, Wo, bo, _trace=False, _trace_kwargs=None):
    if "nc" not in _cache:
        _cache["nc"] = build_nc()
    nc = _cache["nc"]
    in_maps = _prep_inputs(x, Wq, bq, Wk, bk, Wv, bv, Wo, bo)
    res = run_bass_kernel_spmd(nc, in_maps, list(range(N_CORES)),
                               trace=_trace, **(_trace_kwargs or {}))
    _cache["last_results"] = res
    out = np.empty((B, S, D), np.float32)
    for c in range(N_CORES):
        o = res.results[c]["outT"]  # [1024, 512]; cols = 256b + 128hf + tok
        for b in range(B):
            for hf in range(2):
                sl = o[:, TPC * b + 128 * hf:TPC * b + 128 * (hf + 1)]
                out[b, 1024 * hf + 128 * c:1024 * hf + 128 * (c + 1), :] = sl.T
    return out


# revision 37
# speedup vs baseline: 1.0085x; 1.0085x over previous
"""Multi-head attention (B=2, S=2048, D=1024, H=16) on 8 TRN2 NeuronCores.

Sharding: tensor-parallel over heads. Core c owns heads {2c, 2c+1} (feature
columns [128c, 128c+128)). All matmul inputs bf16; psum accumulation fp32.

Per-core plan (engine-balanced around the ACT-engine exp floor):
  - Q^T/K^T projections feature-major [feat, tok]; bk dropped (cancels in
    softmax); 1/sqrt(dh) folded into Wq/bq on host.
  - V projected token-major [tok, feat] directly (no transposes); bv folded
    into the output-projection bias on host (bo' = bo + bv @ Wo).
  - scores per (batch, head): out [128 keys, 1024 q] psum, exp on ACT into
    bf16; attnV in [q, dh] layout (full PE utilization); denominator via
    N=1 matmuls against a ones vector; normalization via per-partition
    reciprocal + tensor_scalar_mul on DVE.
  - Token ownership: core c owns tokens [256c, 256c+256) of EACH batch.
    One AllToAll per batch (launched as soon as that batch's two heads
    finish), receiver-side dma_start_transpose to feature-major, output
    projection per batch overlaps the next batch's attention.
  - PE is in-order, so PE idle gaps during the ACT-bound attention are
    backfilled with fine-grained (per-matmul) filler units: the remaining
    projections and the batch-0 output projection.
"""
import sys
sys.path.insert(0, "/opt/trn_rl_repo")
from collections import deque
from contextlib import ExitStack

import numpy as np

import concourse.bass as bass
import concourse.bacc as bacc
import concourse.mybir as mybir
import concourse.tile as tile
from concourse.bass_utils import run_bass_kernel_spmd

N_CORES = 8
B, S, D = 2, 2048, 1024
T = B * S                  # 4096 tokens
H, DH = 16, 64
F = D // N_CORES           # 128 features per core (2 heads)
TPC = 256                  # tokens owned per (core, batch)
NT = 8                     # 512-token tiles
NKT = 16                   # key tiles of 128 per batch
NCC = 8                    # contraction chunks of 128

F32 = mybir.dt.float32
BF16 = mybir.dt.bfloat16
EXP = mybir.ActivationFunctionType.Exp

_cache = {}
_DEBUG_OT = False


class FillQueue:
    """FIFO of (pe_ns, flags, emit_fn) filler units, drained between attention
    iterations to backfill PE idle. drain_until(flag) force-emits everything
    up to and including the unit that provides `flag`."""

    def __init__(self):
        self.q = deque()
        self.flags = set()

    def add(self, ns, fn, flags=()):
        self.q.append((ns, tuple(flags), fn))

    def _pop(self):
        ns, flags, fn = self.q.popleft()
        fn()
        self.flags.update(flags)
        return ns

    def drain(self, budget_ns):
        while self.q and budget_ns > 0:
            budget_ns -= self._pop()

    def drain_until(self, flag):
        while flag not in self.flags:
            assert self.q, f"filler queue exhausted waiting for {flag}"
            self._pop()

    def drain_all(self):
        while self.q:
            self._pop()


def build_nc():
    nc = bacc.Bacc()
    # host-prepacked layouts (see _prep_inputs)
    x_e = nc.dram_tensor("xh", [128, NT * 4096], BF16, kind="ExternalInput")
    wq_e = nc.dram_tensor("wq", [128, D], BF16, kind="ExternalInput")
    wk_e = nc.dram_tensor("wk", [128, D], BF16, kind="ExternalInput")
    wv_e = nc.dram_tensor("wv", [128, D], BF16, kind="ExternalInput")
    bq_e = nc.dram_tensor("bq", [128, 1], F32, kind="ExternalInput")
    wo_e = nc.dram_tensor("wo", [128, NCC * D], BF16, kind="ExternalInput")
    bo2_e = nc.dram_tensor("bo2", [128, NCC], F32, kind="ExternalInput")
    id_e = nc.dram_tensor("ident", [128, 128], BF16, kind="ExternalInput")
    outT_e = nc.dram_tensor("outT", [D, 2 * TPC], F32, kind="ExternalOutput")
    dbg_e = nc.dram_tensor("dbg", [128, 2 * 2048], BF16,
                           kind="ExternalOutput") if _DEBUG_OT else None
    dbg2_e = nc.dram_tensor("dbg2", [128, 3 * T], BF16,
                            kind="ExternalOutput") if _DEBUG_OT else None

    with tile.TileContext(nc) as tc, ExitStack() as top:
        misc = top.enter_context(tc.tile_pool(name="misc", bufs=1))
        bq_sb = misc.tile([128, 1], F32)
        bo2_sb = misc.tile([128, NCC], F32)
        ones_sb = misc.tile([128, 1], BF16)
        id_sb = misc.tile([128, 128], BF16)
        nc.gpsimd.memset(ones_sb[:], 1.0)

        sb = top.enter_context(tc.tile_pool(name="sb", bufs=1))
        wq_sb = sb.tile([128, D], BF16, tag="wq")
        wk_sb = sb.tile([128, D], BF16, tag="wk")
        wv_sb = sb.tile([128, D], BF16, tag="wv")
        wo_sb = sb.tile([128, NCC * D], BF16, tag="wo")
        Qt = sb.tile([128, T], BF16, tag="Qt")      # [feat, tok]
        Kt = sb.tile([128, T], BF16, tag="Kt")
        xs = [sb.tile([128, 4096], BF16, tag=f"xs{t}", name=f"xs{t}")
              for t in range(NT)]
        V2 = [sb.tile([128, 512], BF16, tag=f"v2_{t}", name=f"v2_{t}")
              for t in range(NT)]                   # [tok128x4, feat]
        OT = [sb.tile([128, 2048], BF16, tag=f"ot{b}", name=f"ot{b}")
              for b in range(B)]                    # [q128, qt*128 + 64h + dh]
        InT = [[sb.tile([128, 1024], BF16, tag=f"in{b}{hf}",
                        name=f"in{b}{hf}") for hf in range(2)]
               for b in range(B)]                   # [feat128, 128cc + tok]
        Atm = [[sb.tile([128, 1024], BF16, tag=f"atm{b}{hf}",
                        name=f"atm{b}{hf}") for hf in range(2)]
               for b in range(B)]                   # token-major recv staging
        atp = top.enter_context(tc.tile_pool(name="atp", bufs=3))
        recp = top.enter_context(tc.tile_pool(name="recp", bufs=2))
        osbp = top.enter_context(tc.tile_pool(name="osbp", bufs=2))

        # PSUM: pair(sc + tail outproj) 2x[128,1024]=4 banks, ov 2, dn 1,
        # b512 (v-proj / interleaved qk-proj / outproj-b0 fillers) 1 => 8
        pairp = top.enter_context(tc.tile_pool(name="pairp", bufs=2, space="PSUM"))
        ovp = top.enter_context(tc.tile_pool(name="ovp", bufs=1, space="PSUM"))
        dnp_pool = top.enter_context(tc.tile_pool(name="dnp", bufs=1, space="PSUM"))
        b512 = top.enter_context(tc.tile_pool(name="b512", bufs=1, space="PSUM"))

        # Each batch's AllToAll is split into two half collectives of
        # [8, 128, 128]: core c owns tokens qt=c (half 0) and qt=8+c (half 1)
        # of each batch, so the lo half is complete as soon as the qh0 sweep
        # of the batch's last head finishes.
        dram = top.enter_context(tc.tile_pool(name="dram", bufs=1, space="DRAM"))
        a2a_in = [[dram.tile([N_CORES, 128, 128], BF16, tag=f"ain{b}{hf}",
                             name=f"ain{b}{hf}") for hf in range(2)]
                  for b in range(B)]
        a2a_out = [[dram.tile([N_CORES, 128, 128], BF16, tag=f"aout{b}{hf}",
                              name=f"aout{b}{hf}") for hf in range(2)]
                   for b in range(B)]

        # input DMAs, readiness-ordered on the SP queue
        nc.sync.dma_start(out=wq_sb[:], in_=wq_e[:])
        nc.sync.dma_start(out=xs[0][:, 0:2048], in_=x_e[:, 0:2048])
        nc.sync.dma_start(out=wk_sb[:], in_=wk_e[:])
        nc.sync.dma_start(out=xs[0][:, 2048:4096], in_=x_e[:, 2048:4096])
        nc.sync.dma_start(out=xs[1][:, 0:2048], in_=x_e[:, 4096:6144])
        nc.sync.dma_start(out=xs[1][:, 2048:4096], in_=x_e[:, 6144:8192])
        nc.sync.dma_start(out=bq_sb[:], in_=bq_e[:])
        nc.sync.dma_start(out=wv_sb[:], in_=wv_e[:])
        nc.sync.dma_start(out=bo2_sb[:], in_=bo2_e[:])
        nc.sync.dma_start(out=id_sb[:], in_=id_e[:])
        for t in range(2, NT):
            nc.sync.dma_start(out=xs[t][:], in_=x_e[:, 4096 * t:4096 * (t + 1)])
        nc.sync.dma_start(out=wo_sb[:], in_=wo_e[:])

        # ---------- projection emitters ----------
        def emit_pair(kind, ta, tb):
            """Blob form (pre-attention only): [128,1024] psum with proj of
            tiles ta,tb."""
            w_sb, dst = (wq_sb, Qt) if kind == "q" else (wk_sb, Kt)
            pt = pairp.tile([128, 1024], F32, tag="pair", name=f"p{kind}{ta}{tb}")
            for i, t in enumerate((ta, tb)):
                for cc in range(NCC):
                    nc.tensor.matmul(
                        pt[:, 512 * i:512 * (i + 1)],
                        w_sb[:, 128 * cc:128 * (cc + 1)],
                        xs[t][:, 512 * cc:512 * (cc + 1)],
                        start=(cc == 0), stop=(cc == NCC - 1))
            for i, t in enumerate((ta, tb)):
                sl = pt[:, 512 * i:512 * (i + 1)]
                if kind == "q":
                    nc.vector.tensor_scalar_add(
                        dst[:, 512 * t:512 * (t + 1)], sl, bq_sb[:])
                else:
                    nc.vector.tensor_copy(dst[:, 512 * t:512 * (t + 1)], sl)

        def add_qk_units(fill, kind, t, flags):
            """Fine-grained filler: Q or K projection of tile t on b512."""
            w_sb, dst = (wq_sb, Qt) if kind == "q" else (wk_sb, Kt)
            box = {}

            def mk(cc):
                def emit():
                    if cc == 0:
                        box["pt"] = b512.tile([128, 512], F32, tag="b512",
                                              name=f"s{kind}{t}")
                    nc.tensor.matmul(
                        box["pt"][:], w_sb[:, 128 * cc:128 * (cc + 1)],
                        xs[t][:, 512 * cc:512 * (cc + 1)],
                        start=(cc == 0), stop=(cc == NCC - 1))
                return emit

            for cc in range(NCC):
                fill.add(430, mk(cc))

            def fin():
                if kind == "q":
                    nc.vector.tensor_scalar_add(
                        dst[:, 512 * t:512 * (t + 1)], box["pt"][:], bq_sb[:])
                else:
                    nc.vector.tensor_copy(dst[:, 512 * t:512 * (t + 1)],
                                          box["pt"][:])
            fill.add(60, fin, flags)

        def add_v_units(fill, t, flags):
            box = {}

            def mk(j):
                def emit():
                    if j == 0:
                        box["vp"] = b512.tile([128, 512], F32, tag="b512",
                                              name=f"vps{t}")
                    for cc in range(NCC):
                        nc.tensor.matmul(
                            box["vp"][:, 128 * j:128 * (j + 1)],
                            xs[t][:, 512 * cc + 128 * j:512 * cc + 128 * (j + 1)],
                            wv_sb[:, 128 * cc:128 * (cc + 1)],
                            start=(cc == 0), stop=(cc == NCC - 1))
                return emit

            for j in range(4):
                fill.add(430, mk(j))

            def fin():
                nc.vector.tensor_copy(V2[t][:], box["vp"][:])
            fill.add(60, fin, flags)

        # ---------- output projection ----------
        # out columns: outT_e[:, 256b + 128hf + tok]; token = 128c + 1024hf
        def _outproj_chunk(b, hf, n, ops, osb):
            for cc in range(NCC):
                nc.tensor.matmul(
                    ops[:, 0:128],
                    wo_sb[:, D * cc + 128 * n:D * cc + 128 * (n + 1)],
                    InT[b][hf][:, 128 * cc:128 * (cc + 1)],
                    start=(cc == 0), stop=(cc == NCC - 1))
            nc.vector.tensor_scalar_add(
                osb[:, 128 * n:128 * (n + 1)], ops[:, 0:128], bo2_sb[:, n:n + 1])

        def _outproj_dma(b, hf, osb, half):
            nc.sync.dma_start(
                out=outT_e[512 * half:512 * (half + 1),
                           256 * b + 128 * hf:256 * b + 128 * (hf + 1)]
                    .rearrange("(n p) f -> p n f", p=128),
                in_=osb[:, 512 * half:512 * (half + 1)]
                    .rearrange("p (n f) -> p n f", n=4))

        def add_outproj_units(q_lo, q_hi, b):
            """Filler form for batch b on the b512 psum tag; hf=0 units go to
            q_lo, hf=1 (gated on the later collective half) to q_hi."""
            for hf, q in ((0, q_lo), (1, q_hi)):
                state = {}

                def mk(n, hf=hf, state=state):
                    def emit():
                        if n == 0:
                            state["osb"] = osbp.tile([128, 1024], F32, tag="osb",
                                                     name=f"osb{b}{hf}")
                        ops = b512.tile([128, 512], F32, tag="b512",
                                        name=f"ops{b}{hf}{n}")
                        _outproj_chunk(b, hf, n, ops, state["osb"])
                        if n % 4 == 3:
                            _outproj_dma(b, hf, state["osb"], n // 4)
                    return emit

                for n in range(NCC):
                    q.add(500, mk(n))

        def emit_outproj_tail(b, hf):
            """Tail form: pair-pool tile per fout chunk (bufs=2 pipelines the
            psum WAR between a chunk's bias-add read and the next's matmuls)."""
            osb = osbp.tile([128, 1024], F32, tag="osbq", name=f"osbq{b}{hf}")
            for n in range(NCC):
                ops = pairp.tile([128, 1024], F32, tag="pair",
                                 name=f"opc{b}{hf}{n}")
                _outproj_chunk(b, hf, n, ops, osb)
                if n % 4 == 3:
                    _outproj_dma(b, hf, osb, n // 4)

        # ---------- attention block ----------
        def attention_block(b, h, fill: FillQueue, carry=None, mid_extra=None,
                            late_fill=None, budget_ns=420):
            """Emits scores+exp+attnV for (b, h). The final attnV flush and
            normalization are NOT emitted here; they are returned as a
            `finish` closure which the NEXT block runs (via `carry`) right
            after its first exp, so the next block's scores are already in
            flight on the in-order PE before the flush/norm chain."""
            hs = slice(64 * h, 64 * (h + 1))
            ov = ovp.tile([128, 1024], F32, tag="ov", name=f"ov{b}{h}")
            dn = dnp_pool.tile([128, 16], F32, tag="dn", name=f"dn{b}{h}")
            pending = None
            first = True

            def emit_attnv(kt, qh, at):
                # The simulator zeroes psum lazily at ZERO-REGION (2KB bank)
                # granularity on start=True, so each bank must be ONE
                # accumulation group: start only on the bank's first write
                # (later regions read pending-zero as 0), stop on its last.
                fill.drain_until(f"v{b}t{kt // 4}")
                t = 4 * b + kt // 4
                j = kt % 4
                for i in range(8):
                    qt = 8 * qh + i
                    nc.tensor.matmul(
                        ov[:, 64 * qt:64 * (qt + 1)],
                        at[:, 128 * i:128 * (i + 1)],
                        V2[t][:, 128 * j + 64 * h:128 * j + 64 * h + 64],
                        start=(kt == 0 and i == 0),
                        stop=(kt == NKT - 1 and i == 7),
                        skip_group_check=True)
                    nc.tensor.matmul(
                        dn[:, qt:qt + 1],
                        at[:, 128 * i:128 * (i + 1)],
                        ones_sb[:, 0:1],
                        start=(qh == 0 and kt == 0 and i == 0),
                        stop=(qh == 1 and kt == NKT - 1 and i == 7),
                        skip_group_check=True)

            rc = recp.tile([128, 16], F32, tag="rc", name=f"rc{b}{h}")

            def norm_half(hf):
                # normalize qt in [8hf, 8hf+8): reciprocal + broadcast multiply
                nc.vector.reciprocal(rc[:, 8 * hf:8 * (hf + 1)],
                                     dn[:, 8 * hf:8 * (hf + 1)])
                nc.vector.tensor_mul(
                    OT[b][:, 1024 * hf:1024 * (hf + 1)]
                        .rearrange("p (qt hh d) -> p qt hh d",
                                   hh=2, d=64)[:, :, h, :],
                    ov[:, 512 * hf:512 * (hf + 1)]
                        .rearrange("p (qt d) -> p qt d", d=64),
                    rc[:, 8 * hf:8 * (hf + 1)].to_broadcast([128, 8, 64]))

            for qh in range(2):
                fill.drain_until(f"q{b}{'lo' if qh == 0 else 'hi'}")
                for kt in range(NKT):
                    fill.drain_until(f"k{b}t{kt // 4}")
                    sc = pairp.tile([128, 1024], F32, tag="pair",
                                    name=f"sc{b}{h}{qh}{kt}")
                    for i in range(2):
                        q0 = 2048 * b + 1024 * qh + 512 * i
                        nc.tensor.matmul(
                            sc[:, 512 * i:512 * (i + 1)],
                            Kt[hs, 2048 * b + 128 * kt:2048 * b + 128 * (kt + 1)],
                            Qt[hs, q0:q0 + 512],
                            start=True, stop=True)
                    at = atp.tile([128, 1024], BF16, tag="at",
                                  name=f"at{b}{h}{qh}{kt}")
                    nc.scalar.activation(at[:], sc[:], EXP)
                    if first:
                        if carry is not None:
                            carry()
                        first = False
                    if pending is not None:
                        emit_attnv(*pending)
                        if pending[0] == NKT - 1 and pending[1] == 0:
                            # qh0 sweep flushed: lo-half norm (and a2a hooks)
                            norm_half(0)
                            if mid_extra is not None:
                                mid_extra()
                    pending = (kt, qh, at)
                    fill.drain(budget_ns)
                    if late_fill is not None and qh == 1:
                        late_fill.drain(budget_ns)

            def finish():
                emit_attnv(*pending)
                norm_half(1)
            return finish

        def emit_stage(b, hf):
            nc.sync.dma_start(
                out=a2a_in[b][hf][:].rearrange("r p f -> p r f"),
                in_=OT[b][:, 1024 * hf:1024 * (hf + 1)]
                    .rearrange("p (r f) -> p r f", r=N_CORES))

        def emit_coll(b, hf):
            nc.gpsimd.collective_compute(
                "AllToAll", mybir.AluOpType.bypass,
                ins=[a2a_in[b][hf][:].opt()],
                outs=[a2a_out[b][hf][:].opt()],
                replica_groups=[list(range(N_CORES))])

        def emit_recv_load(b, hf, half):
            # token-major load of 4 source chunks: Atm[p=tok, 128r + f]
            rs = slice(4 * half, 4 * (half + 1))
            nc.sync.dma_start(
                out=Atm[b][hf][:, 512 * half:512 * (half + 1)]
                    .rearrange("p (r f) -> p r f", r=4),
                in_=a2a_out[b][hf][rs, :, :].rearrange("r t f -> t r f"))

        def emit_recv_transpose(b, hf, half):
            # PE transposes to feature-major InT[b][hf] (cols = 128cc + tok).
            # (dma_start_transpose is serialized against collectives by the
            # tile framework, so transpose on the PE instead.)
            if half == 0:
                recv_tp[(b, hf)] = pairp.tile([128, 1024], BF16, tag="pair",
                                              name=f"tp{b}{hf}")
            tp = recv_tp[(b, hf)]
            for r in range(4 * half, 4 * (half + 1)):
                nc.tensor.transpose(tp[:, 128 * r:128 * (r + 1)],
                                    Atm[b][hf][:, 128 * r:128 * (r + 1)],
                                    id_sb[:])
            nc.vector.tensor_copy(
                InT[b][hf][:, 512 * half:512 * (half + 1)],
                tp[:, 512 * half:512 * (half + 1)])

        recv_tp = {}

        def add_recv_units(q, b, hf):
            for half in range(2):
                q.add(100, lambda hx=half: emit_recv_load(b, hf, hx))
                q.add(400, lambda hx=half: emit_recv_transpose(b, hf, hx))

        # ================= schedule =================
        fill = FillQueue()
        fill.flags.update({"q0lo", "k0t0", "k0t1"})
        # pre-phase: one psum tile per (tensor, t); t0 first (xs0 halves),
        # t1 matmuls hide behind the xs1 DMA
        for t in (0, 1):
            for kind, w_sb in (("q", wq_sb), ("k", wk_sb)):
                pt = pairp.tile([128, 1024], F32, tag="pair",
                                name=f"pre{kind}{t}")
                for cc in range(NCC):
                    nc.tensor.matmul(
                        pt[:, 0:512],
                        w_sb[:, 128 * cc:128 * (cc + 1)],
                        xs[t][:, 512 * cc:512 * (cc + 1)],
                        start=(cc == 0), stop=(cc == NCC - 1))
                if kind == "q":
                    nc.vector.tensor_scalar_add(
                        Qt[:, 512 * t:512 * (t + 1)], pt[:, 0:512], bq_sb[:])
                else:
                    nc.vector.tensor_copy(
                        Kt[:, 512 * t:512 * (t + 1)], pt[:, 0:512])

        add_v_units(fill, 0, ["v0t0"])
        add_v_units(fill, 1, ["v0t1"])
        add_qk_units(fill, "k", 2, ["k0t2"])
        add_v_units(fill, 2, ["v0t2"])
        add_qk_units(fill, "k", 3, ["k0t3"])
        add_v_units(fill, 3, ["v0t3"])
        add_qk_units(fill, "q", 2, [])
        add_qk_units(fill, "q", 3, ["q0hi"])
        add_qk_units(fill, "q", 4, [])
        add_qk_units(fill, "q", 5, ["q1lo"])
        add_qk_units(fill, "k", 4, ["k1t0"])
        add_v_units(fill, 4, ["v1t0"])
        add_qk_units(fill, "k", 5, ["k1t1"])
        add_v_units(fill, 5, ["v1t1"])
        add_qk_units(fill, "k", 6, ["k1t2"])
        add_v_units(fill, 6, ["v1t2"])
        add_qk_units(fill, "k", 7, ["k1t3"])
        add_v_units(fill, 7, ["v1t3"])
        add_qk_units(fill, "q", 6, [])
        add_qk_units(fill, "q", 7, ["q1hi"])

        f00 = attention_block(0, 0, fill)

        def mid01():   # b0 lo-half complete once b0h1's qh0 sweep is normed
            emit_stage(0, 0)
            emit_coll(0, 0)
        f01 = attention_block(0, 1, fill, carry=f00, mid_extra=mid01)

        def carry10():
            f01()
            emit_stage(0, 1)
            emit_coll(0, 1)
        f10 = attention_block(1, 0, fill, carry=carry10)
        late = FillQueue()
        add_recv_units(fill, 0, 0)
        add_recv_units(late, 0, 1)
        add_outproj_units(fill, late, 0)

        def mid11():
            emit_stage(1, 0)
            emit_coll(1, 0)
        f11 = attention_block(1, 1, fill, carry=f10, mid_extra=mid11,
                              late_fill=late)
        fill.drain_all()
        late.drain_all()
        f11()
        emit_stage(1, 1)
        emit_coll(1, 1)
        for hf in range(2):
            for half in range(2):
                emit_recv_load(1, hf, half)
                emit_recv_transpose(1, hf, half)
            emit_outproj_tail(1, hf)
        if _DEBUG_OT:
            for b in range(B):
                nc.sync.dma_start(out=dbg_e[:, 2048 * b:2048 * (b + 1)],
                                  in_=OT[b][:])
            nc.sync.dma_start(out=dbg2_e[:, 0:T], in_=Qt[:])
            nc.sync.dma_start(out=dbg2_e[:, T:2 * T], in_=Kt[:])
            for t in range(NT):
                nc.sync.dma_start(
                    out=dbg2_e[:, 2 * T + 512 * t:2 * T + 512 * (t + 1)],
                    in_=V2[t][:])

    nc.finalize()
    return nc


def _prep_inputs(x, Wq, bq, Wk, bk, Wv, bv, Wo, bo):
    import ml_dtypes
    bf16 = ml_dtypes.bfloat16
    scale = 1.0 / np.sqrt(DH)

    xf = np.asarray(x, np.float32).reshape(T, D)
    # xh[p, 4096t + 512cc + f] = xf[512t + f, 128cc + p]
    xh = np.ascontiguousarray(
        xf.reshape(NT, 512, NCC, 128).transpose(3, 0, 2, 1).reshape(128, NT * 4096)
    ).astype(bf16)

    def pack_w(W):  # [1024, 128] -> [128, 1024]: out[p, 128cc+f] = W[128cc+p, f]
        return np.ascontiguousarray(
            np.asarray(W, np.float32).reshape(NCC, 128, 128)
            .transpose(1, 0, 2).reshape(128, D))

    Wo64 = np.asarray(Wo, np.float64)
    bo_f = (np.asarray(bo, np.float64) +
            np.asarray(bv, np.float64) @ Wo64).astype(np.float32)
    bo2 = np.ascontiguousarray(bo_f.reshape(NCC, 128).T)
    # wo_sb[p, 1024cc + fo] = Wo[128cc + p, fo]
    wo_p = np.ascontiguousarray(
        np.asarray(Wo, np.float32).reshape(NCC, 128, D)
        .transpose(1, 0, 2).reshape(128, NCC * D)).astype(bf16)
    ident = np.eye(128, dtype=np.float32).astype(bf16)

    in_maps = []
    for c in range(N_CORES):
        fs = slice(F * c, F * (c + 1))
        in_maps.append({
            "xh": xh,
            "wq": pack_w(np.asarray(Wq, np.float32)[:, fs] * scale).astype(bf16),
            "wk": pack_w(np.asarray(Wk, np.float32)[:, fs]).astype(bf16),
            "wv": pack_w(np.asarray(Wv, np.float32)[:, fs]).astype(bf16),
            "bq": np.ascontiguousarray(
                (np.asarray(bq, np.float32)[fs] * scale)[:, None]),
            "wo": wo_p,
            "bo2": bo2,
            "ident": ident,
        })
    return in_maps


def kernel(x, Wq, bq, Wk, bk, Wv, bv, Wo, bo, _trace=False, _trace_kwargs=None):
    if "nc" not in _cache:
        _cache["nc"] = build_nc()
    nc = _cache["nc"]
    in_maps = _prep_inputs(x, Wq, bq, Wk, bk, Wv, bv, Wo, bo)
    res = run_bass_kernel_spmd(nc, in_maps, list(range(N_CORES)),
                               trace=_trace, **(_trace_kwargs or {}))
    _cache["last_results"] = res
    out = np.empty((B, S, D), np.float32)
    for c in range(N_CORES):
        o = res.results[c]["outT"]  # [1024, 512]; cols = 256b + 128hf + tok
        for b in range(B):
            for hf in range(2):
                sl = o[:, TPC * b + 128 * hf:TPC * b + 128 * (hf + 1)]
                out[b, 1024 * hf + 128 * c:1024 * hf + 128 * (c + 1), :] = sl.T
    return out
